# revision 13
# baseline (speedup 1.0000x reference)
"""Trainium2 Bass kernel for nn_DatTransformer (sparse hard-max attention).

Sharding: 8 cores = 4 batches x 2 query-halves. Key algebra: with q = x Wq^T + bq
and k = x Wk^T + bk, the argmax-relevant part of scores is (x M + w_c) x^T where
M = Wq^T Wk and w_c = Wk^T bq (the row-constant term x_s Wq^T bk + bq.bk does not
affect argmax and is restored only for the threshold test). So the kernel computes
a single projection u = x M + w_c per query row, then scores = u x^T against raw
(pre-split) x. Precision: hi terms on the 12-bit float32r grid, cross terms
(u_hi.x_lo + u_lo.x_hi) in fp8e4 DoubleRow matmuls with lo pre-scaled by 2^13 and
the hi score operand scaled by 2^13 so all passes share one PSUM (argmax is
scale-invariant). Winning rows are fetched by indirect DMA of raw x and pushed
through the fused (v_w.T @ out_w.T) projection.
"""
import sys, os

for _p in ("/root/.axon_site", "/root/.axon_site/_ro/trn_rl_repo",
           "/root/.axon_site/_ro/pypackages", "/opt/trn_rl_repo"):
    if os.path.isdir(_p) and _p not in sys.path:
        sys.path.append(_p)

import numpy as np
import concourse.bass as bass
import concourse.bacc as bacc
import concourse.mybir as mybir
from concourse.tile import TileContext
from concourse.bass_utils import run_bass_kernel_spmd
from concourse import masks

P = 128
S = 4096          # keys per batch
SQ = 2048         # queries per core
D = 512
NE = D // P       # 4 contraction chunks
NQT = SQ // P     # 16 query tiles
KB = 512          # k-block width for the score phase
NKB = S // KB     # 8 k-blocks
THRESH = 0.95
SCALE = 8192.0    # 2^13 lo-term scale

F32 = mybir.dt.float32
F32R = mybir.dt.float32r
F8 = mybir.dt.float8e4
U32 = mybir.dt.uint32
AF = mybir.ActivationFunctionType
ALU = mybir.AluOpType
DR = mybir.MatmulPerfMode.DoubleRow

_CACHED = {}


def round_f32r(a: np.ndarray) -> np.ndarray:
    """Round fp32 array to the 12-explicit-mantissa-bit float32r grid (RNE)."""
    b = np.ascontiguousarray(a, dtype=np.float32).view(np.uint32)
    r = (b + 0x7FF + ((b >> 12) & 1)) & np.uint32(0xFFFFF000)
    return r.view(np.float32).copy()


def build_nc(repeat: int = 1):
    nc = bacc.Bacc("TRN2", target_bir_lowering=False, debug=False, num_devices=8)

    xT_hi = nc.declare_dram_parameter("xT_hi", [D, S], F32R, isOutput=False)
    xh8_a = nc.declare_dram_parameter("xh8_a", [P, 2 * S], F8, isOutput=False)
    xh8_b = nc.declare_dram_parameter("xh8_b", [P, 2 * S], F8, isOutput=False)
    xl8_a = nc.declare_dram_parameter("xl8_a", [P, 2 * S], F8, isOutput=False)
    xl8_b = nc.declare_dram_parameter("xl8_b", [P, 2 * S], F8, isOutput=False)
    m_hi = nc.declare_dram_parameter("m_hi", [D, D], F32R, isOutput=False)
    m8h_a = nc.declare_dram_parameter("m8h_a", [P, 2 * D], F8, isOutput=False)
    m8h_b = nc.declare_dram_parameter("m8h_b", [P, 2 * D], F8, isOutput=False)
    m8l_a = nc.declare_dram_parameter("m8l_a", [P, 2 * D], F8, isOutput=False)
    m8l_b = nc.declare_dram_parameter("m8l_b", [P, 2 * D], F8, isOutput=False)
    wc = nc.declare_dram_parameter("wc", [D], F32, isOutput=False)
    w2 = nc.declare_dram_parameter("w2", [D], F32R, isOutput=False)
    cb = nc.declare_dram_parameter("cb", [P, 1], F32, isOutput=False)
    xg_src = nc.declare_dram_parameter("xg_src", [S, D], F32, isOutput=False)
    wvo = nc.declare_dram_parameter("wvo", [D, D], F32R, isOutput=False)
    bvo_row = nc.declare_dram_parameter("bvo_row", [1, D], F32R, isOutput=False)
    ob_bcast = nc.declare_dram_parameter("ob_bcast", [P, D], F32, isOutput=False)
    out_d = nc.declare_dram_parameter("out", [SQ, D], F32, isOutput=True)

    with TileContext(nc) as tc:
        with tc.tile_pool(name="resident", bufs=1) as rp, \
             tc.tile_pool(name="mm", bufs=4, space="PSUM") as mmp, \
             tc.tile_pool(name="tp", bufs=2, space="PSUM") as tpp, \
             tc.tile_pool(name="op", bufs=2, space="PSUM") as opp, \
             tc.tile_pool(name="stats", bufs=1) as stp, \
             tc.tile_pool(name="fin", bufs=2) as fp:

            # ---- resident loads ----
            xt = [rp.tile([P, S], F32R, name=f"xt{e}", tag=f"xt{e}") for e in range(NE)]
            for e in range(NE):
                nc.sync.dma_start(out=xt[e][:], in_=xT_hi[e * P:(e + 1) * P, :])
            xh8 = [rp.tile([P, 2, S], F8, name=f"xh8{i}", tag=f"xh8{i}") for i in range(2)]
            xl8 = [rp.tile([P, 2, S], F8, name=f"xl8{i}", tag=f"xl8{i}") for i in range(2)]
            for i, (dsth, dstl, srch, srcl) in enumerate(
                    [(xh8[0], xl8[0], xh8_a, xl8_a), (xh8[1], xl8[1], xh8_b, xl8_b)]):
                nc.sync.dma_start(out=dsth[:], in_=srch.rearrange("p (t s) -> p t s", t=2))
                nc.sync.dma_start(out=dstl[:], in_=srcl.rearrange("p (t s) -> p t s", t=2))
            mhi_t = [rp.tile([P, D], F32R, name=f"mhi{d}", tag=f"mhi{d}") for d in range(NE)]
            for d in range(NE):
                nc.sync.dma_start(out=mhi_t[d][:], in_=m_hi[d * P:(d + 1) * P, :])
            m8h = [rp.tile([P, 2, D], F8, name=f"m8h{i}", tag=f"m8h{i}") for i in range(2)]
            m8l = [rp.tile([P, 2, D], F8, name=f"m8l{i}", tag=f"m8l{i}") for i in range(2)]
            for i, (src_h, src_l) in enumerate([(m8h_a, m8l_a), (m8h_b, m8l_b)]):
                nc.sync.dma_start(out=m8h[i][:], in_=src_h.rearrange("p (t s) -> p t s", t=2))
                nc.sync.dma_start(out=m8l[i][:], in_=src_l.rearrange("p (t s) -> p t s", t=2))
            wc_t = [rp.tile([P, 1], F32, name=f"wc{e}", tag=f"wc{e}") for e in range(NE)]
            wc_r = wc.rearrange("(e p) -> e p", p=P)
            for e in range(NE):
                nc.sync.dma_start(out=wc_t[e][:, 0], in_=wc_r[e])
            w2_t = rp.tile([P, NE], F32R, name="w2t")
            w2_r = w2.rearrange("(e p) -> e p", p=P)
            for e in range(NE):
                nc.sync.dma_start(out=w2_t[:, e], in_=w2_r[e])
            cb_t = rp.tile([P, 1], F32, name="cbt")
            nc.sync.dma_start(out=cb_t[:], in_=cb[:])
            wvo_t = [rp.tile([P, D], F32R, name=f"wvo{d}", tag=f"wvo{d}") for d in range(NE)]
            for d in range(NE):
                nc.sync.dma_start(out=wvo_t[d][:], in_=wvo[d * P:(d + 1) * P, :])
            bvo_t = rp.tile([1, D], F32R, name="bvo_t")
            nc.sync.dma_start(out=bvo_t[:], in_=bvo_row[:])
            ob_t = rp.tile([P, D], F32, name="ob_t")
            nc.sync.dma_start(out=ob_t[:], in_=ob_bcast[:])
            ident = rp.tile([P, P], F32, name="ident")
            masks.make_identity(nc, ident[:])

            # u-side tiles produced on device
            uhs = [rp.tile([P, SQ], F32R, name=f"uhs{e}", tag=f"uhs{e}") for e in range(NE)]
            uh8 = [rp.tile([P, 2, SQ], F8, name=f"uh8{i}", tag=f"uh8{i}") for i in range(2)]
            ul8 = [rp.tile([P, 2, SQ], F8, name=f"ul8{i}", tag=f"ul8{i}") for i in range(2)]
            t2sb = rp.tile([P, NQT], F32, name="t2sb")
            bmax = [stp.tile([P, NKB], F32, name=f"bmax{q}", tag=f"bmax{q}") for q in range(NQT)]
            bidx = [stp.tile([P, NKB], F32, name=f"bidx{q}", tag=f"bidx{q}") for q in range(NQT)]
            for q in range(NQT):
                nc.vector.memset(bmax[q][:], 0.0)

            from contextlib import nullcontext

            def rep_loop():
                return tc.For_i(0, repeat, 1) if repeat > 1 else nullcontext()

            # ---------------- Phase A: u-projection + t2 ----------------
            with tc.tile_pool(name="pta", bufs=2) as pa:
                with rep_loop():
                    for sc in range(SQ // D):
                        cs = slice(sc * D, (sc + 1) * D)
                        for e in range(NE):
                            es = slice(e * P, (e + 1) * P)
                            ph = mmp.tile([P, D], F32, name="ph", tag="ps")
                            for d in range(NE):
                                nc.tensor.matmul(ph[:], mhi_t[d][:, es], xt[d][:, cs],
                                                 start=(d == 0), stop=(d == NE - 1))
                            pc = mmp.tile([P, D], F32, name="pc", tag="ps")
                            nc.tensor.matmul(pc[:], m8l[0][:, :, es], xh8[0][:, :, cs],
                                             start=True, stop=False, perf_mode=DR)
                            nc.tensor.matmul(pc[:], m8l[1][:, :, es], xh8[1][:, :, cs],
                                             start=False, stop=False, perf_mode=DR)
                            nc.tensor.matmul(pc[:], m8h[0][:, :, es], xl8[0][:, :, cs],
                                             start=False, stop=False, perf_mode=DR)
                            nc.tensor.matmul(pc[:], m8h[1][:, :, es], xl8[1][:, :, cs],
                                             start=False, stop=True, perf_mode=DR)
                            # m_hi and wc arrive pre-scaled by 2^13, so tq holds
                            # 2^13*u and uhs is directly the scaled f32r operand.
                            t = pa.tile([P, D], F32, name="t", tag="t")
                            nc.scalar.activation(t[:], pc[:], AF.Identity,
                                                 bias=wc_t[e][:])
                            tq = pa.tile([P, D], F32, name="tq", tag="tq")
                            nc.vector.tensor_add(tq[:], t[:], ph[:])
                            nc.scalar.activation(uhs[e][:, cs], tq[:], AF.Copy)
                            pair, mem = divmod(e, 2)
                            nc.scalar.activation(uh8[pair][:, mem, cs],
                                                 uhs[e][:, cs], AF.Copy,
                                                 scale=1.0 / SCALE)
                            ulo = pa.tile([P, D], F32, name="ulo", tag="ulo")
                            nc.vector.tensor_sub(ulo[:], tq[:],
                                                 uhs[e][:, cs].bitcast(F32))
                            nc.scalar.activation(ul8[pair][:, mem, cs], ulo[:],
                                                 AF.Copy)
                    # t2[s] = x_s . (Wq^T bk) + bq.bk  (restores row-constant term)
                    for q in range(NQT):
                        qs = slice(q * P, (q + 1) * P)
                        pt2 = tpp.tile([P, P], F32, name="pt2", tag="pt")
                        for d in range(NE):
                            nc.tensor.matmul(pt2[:, 0:1], xt[d][:, qs].bitcast(F32),
                                             w2_t[:, d:d + 1].bitcast(F32),
                                             start=(d == 0), stop=(d == NE - 1))
                        nc.scalar.activation(t2sb[:, q:q + 1], pt2[:, 0:1], AF.Identity,
                                             bias=cb_t[:])

            # ---------------- Phase B: scores + argmax + gather ----------------
            def finalize(q):
                qs = slice(q * P, (q + 1) * P)
                gmax = fp.tile([P, 1], F32, name="gmax", tag="gmax")
                nc.vector.tensor_reduce(gmax[:], bmax[q][:], op=ALU.max,
                                        axis=mybir.AxisListType.X)
                idxf = fp.tile([P, 1], F32, name="idxf", tag="idxf")
                nc.vector.tensor_copy(idxf[:], bidx[q][:, NKB - 1:NKB])
                for i in range(NKB - 2, -1, -1):
                    cmpm = fp.tile([P, 1], mybir.dt.uint8, name="cmpm", tag="cmpm")
                    nc.vector.tensor_tensor(cmpm[:], bmax[q][:, i:i + 1], gmax[:],
                                            op=ALU.is_ge)
                    nc.vector.copy_predicated(idxf[:], cmpm[:], bidx[q][:, i:i + 1])
                idxu = fp.tile([P, 1], U32, name="idxu", tag="idxu")
                nc.vector.tensor_copy(idxu[:], idxf[:])
                # sel = (gmax/2^13 + t2) >= THRESH
                gm2 = fp.tile([P, 1], F32, name="gm2", tag="gm2")
                nc.vector.tensor_scalar(gm2[:], gmax[:], 1.0 / SCALE, None,
                                        op0=ALU.mult)
                gm3 = fp.tile([P, 1], F32, name="gm3", tag="gm3")
                nc.vector.tensor_add(gm3[:], gm2[:], t2sb[:, q:q + 1])
                sel = fp.tile([P, 1], F32, name="sel", tag="sel")
                nc.vector.tensor_scalar(sel[:], gm3[:], float(THRESH), None,
                                        op0=ALU.is_ge)
                xg = fp.tile([P, D], F32, name="xg", tag="xg")
                nc.gpsimd.indirect_dma_start(
                    out=xg[:], out_offset=None, in_=xg_src[:],
                    in_offset=bass.IndirectOffsetOnAxis(ap=idxu[:, :1], axis=0))
                xgm = fp.tile([P, D], F32, name="xgm", tag="xgm")
                nc.gpsimd.tensor_scalar_mul(xgm[:], xg[:], sel[:])
                xgt = []
                for dch in range(NE):
                    pt = tpp.tile([P, P], F32, name="pt", tag="pt")
                    nc.tensor.transpose(pt[:], xgm[:, dch * P:(dch + 1) * P], ident[:])
                    xt_ = fp.tile([P, P], F32R, name=f"xgt{dch}", tag=f"xgt{dch}")
                    nc.scalar.activation(xt_[:], pt[:], AF.Copy)
                    xgt.append(xt_)
                pt2 = tpp.tile([P, P], F32, name="pt2f", tag="pt")
                nc.tensor.transpose(pt2[:1, :], sel[:, :1], ident[:])
                bl = fp.tile([1, P], F32R, name="bl", tag="bl")
                nc.scalar.activation(bl[:, :], pt2[0:1, :], AF.Copy)
                po = opp.tile([P, D], F32, name="po", tag="po")
                for dch in range(NE):
                    nc.tensor.matmul(po[:], xgt[dch][:], wvo_t[dch][:],
                                     start=(dch == 0), stop=False)
                nc.tensor.matmul(po[:], bl[:, :], bvo_t[:], start=False, stop=True)
                outt = fp.tile([P, D], F32, name="outt", tag="outt")
                nc.vector.tensor_add(outt[:], po[:], ob_t[:])
                nc.sync.dma_start(out=out_d[qs, :], in_=outt[:])

            with rep_loop():
                for q in range(NQT):
                    qs = slice(q * P, (q + 1) * P)
                    for kb in range(NKB):
                        ks = slice(kb * KB, (kb + 1) * KB)
                        ps = mmp.tile([P, KB], F32, name="ps", tag="ps")
                        for e in range(NE):
                            nc.tensor.matmul(ps[:], uhs[e][:, qs], xt[e][:, ks],
                                             start=(e == 0), stop=False)
                        nc.tensor.matmul(ps[:], uh8[0][:, :, qs], xl8[0][:, :, ks],
                                         start=False, stop=False, perf_mode=DR)
                        nc.tensor.matmul(ps[:], uh8[1][:, :, qs], xl8[1][:, :, ks],
                                         start=False, stop=False, perf_mode=DR)
                        nc.tensor.matmul(ps[:], ul8[0][:, :, qs], xh8[0][:, :, ks],
                                         start=False, stop=False, perf_mode=DR)
                        nc.tensor.matmul(ps[:], ul8[1][:, :, qs], xh8[1][:, :, ks],
                                         start=False, stop=True, perf_mode=DR)
                        nc.vector.tensor_reduce(bmax[q][:, kb:kb + 1], ps[:],
                                                op=ALU.max,
                                                axis=mybir.AxisListType.X)
                        ix8 = fp.tile([P, 8], U32, name="ix8", tag="ix8")
                        nc.vector.max_index(out=ix8[:], in_max=bmax[q][:, 0:NKB],
                                            in_values=ps[:])
                        ixf = fp.tile([P, 1], F32, name="ixf", tag="ixf")
                        nc.vector.tensor_copy(ixf[:], ix8[:, kb:kb + 1])
                        nc.vector.tensor_scalar_add(bidx[q][:, kb:kb + 1], ixf[:],
                                                    float(kb * KB))
                    # deferred: finalize(q-1) trails qtile q's matmuls in the PE
                    # queue, so the gather/DVE chain never stalls the PE
                    if q > 0:
                        finalize(q - 1)
                finalize(NQT - 1)

    nc.compile()
    return nc


def _get_nc(repeat: int = 1):
    key = ("nc", repeat)
    if key not in _CACHED:
        _CACHED[key] = build_nc(repeat)
    return _CACHED[key]


def _prep_inputs(x, q_w, q_b, k_w, k_b, v_w, v_b, out_w, out_b):
    import ml_dtypes
    f8 = ml_dtypes.float8_e4m3

    M = (q_w.T.astype(np.float64) @ k_w.astype(np.float64)).astype(np.float32)
    w_c = (k_w.T.astype(np.float64) @ q_b.astype(np.float64)).astype(np.float32)
    w2v = (q_w.T.astype(np.float64) @ k_b.astype(np.float64)).astype(np.float32)
    const = np.float32(q_b.astype(np.float64) @ k_b.astype(np.float64))
    Mhi = round_f32r(M)
    Mlo = (M - Mhi).astype(np.float32)
    Mhi8 = Mhi.astype(f8)
    Mlo8 = (Mlo * SCALE).astype(f8)
    Mhi = Mhi * np.float32(SCALE)     # ship 2^13*Mhi: u psum arrives pre-scaled
    w_c = w_c * np.float32(SCALE)

    def pair(a):
        # [D, N] -> two [P, 2*N] chunk-pair arrays ((0,1), (2,3))
        return (np.concatenate([a[0:P], a[P:2 * P]], axis=1),
                np.concatenate([a[2 * P:3 * P], a[3 * P:4 * P]], axis=1))

    m8h_a, m8h_b = pair(Mhi8)
    m8l_a, m8l_b = pair(Mlo8)

    wvo = round_f32r((v_w.T.astype(np.float64) @ out_w.T.astype(np.float64))
                     .astype(np.float32))
    bvo = (v_b.astype(np.float64) @ out_w.T.astype(np.float64)).astype(np.float32)
    bvo_row = round_f32r(bvo[None, :])
    ob = np.tile(out_b.astype(np.float32)[None, :], (P, 1))
    cbv = np.full((P, 1), const, dtype=np.float32)
    w2hi = round_f32r(w2v)

    in_maps = []
    for core in range(8):
        b, h = core // 2, core % 2
        xb = np.ascontiguousarray(x[:, b, :])                    # [S, D]
        order = np.r_[h * SQ:(h + 1) * SQ, (1 - h) * SQ:(2 - h) * SQ]
        xr = np.ascontiguousarray(xb[order])                     # rolled [S, D]
        xT = np.ascontiguousarray(xr.T)                          # [D, S]
        xT_h = round_f32r(xT)
        xT_l = (xT - xT_h).astype(np.float32)
        xh8f = xT_h.astype(f8)
        xl8f = (xT_l * SCALE).astype(f8)
        xh8a, xh8b = pair(xh8f)
        xl8a, xl8b = pair(xl8f)
        in_maps.append({
            "xT_hi": xT_h, "xh8_a": xh8a, "xh8_b": xh8b,
            "xl8_a": xl8a, "xl8_b": xl8b,
            "m_hi": Mhi, "m8h_a": m8h_a, "m8h_b": m8h_b,
            "m8l_a": m8l_a, "m8l_b": m8l_b,
            "wc": w_c, "w2": w2hi, "cb": cbv, "xg_src": xr,
            "wvo": wvo, "bvo_row": bvo_row, "ob_bcast": ob,
        })
    return in_maps


def kernel(x, q_w, q_b, k_w, k_b, v_w, v_b, out_w, out_b, _trace=False,
           **trace_kwargs):
    x, q_w, q_b, k_w, k_b, v_w, v_b, out_w, out_b = (
        np.asarray(a, dtype=np.float32)
        for a in (x, q_w, q_b, k_w, k_b, v_w, v_b, out_w, out_b))
    nc = _get_nc()
    in_maps = _prep_inputs(x, q_w, q_b, k_w, k_b, v_w, v_b, out_w, out_b)
    res = run_bass_kernel_spmd(nc, in_maps, list(range(8)), trace=_trace,
                               **trace_kwargs)
    out = np.empty((S, 4, D), dtype=np.float32)
    for core in range(8):
        b, h = core // 2, core % 2
        out[h * SQ:(h + 1) * SQ, b, :] = res.results[core]["out"]
    if _trace:
        _CACHED["last_results"] = res
    return out
